# revision 1
# baseline (speedup 1.0000x reference)
"""MoE routing kernel for Trainium2 (8 NeuronCores).

Problem: out[b,l,:] = actions[b,l,:]              if action_type[b,l] == 0
         out[b,l,:] = W[t-1] @ actions[b,l,:] + b[t-1]   if action_type == t >= 1

Strategy (type-parallel): instead of computing all 7 expert projections for
every token (the dense reference), route each token to the single expert it
needs. The host groups the B*L tokens by action_type; core t (t=1..7)
processes the tokens of expert t-1 as one dense [C, D] x [D, D] matmul.
Identity-type tokens are copied on the host (exact); core 0 runs the same
SPMD program on zero inputs with W = I and its output is discarded.
Token data is packed transposed ([D, C], feature on partition axis) so the
device does pure dense streaming matmuls - no gather/transpose on device.

Per-core work: ~(B*L/8) tokens * 2*D^2 flops (fp32r ~= full PE rate) and
~4MB weights + 2*C*4KB activations of HBM traffic - right at the
compute/memory ridge.
"""

import sys

for _p in ("/root/.axon_site/_ro/trn_rl_repo", "/opt/trn_rl_repo"):
    if _p not in sys.path:
        sys.path.append(_p)

import numpy as np
import concourse.bass as bass
import concourse.tile as tile
from concourse import bacc, mybir
from concourse.bass_utils import run_bass_kernel_spmd

D = 1024
P = 128
N_CORES = 8
TT = 512  # token tile (moving-dim block)
F32 = mybir.dt.float32
F32R = mybir.dt.float32r

_program_cache: dict[int, bass.Bass] = {}


def build_program(C: int, with_bias: bool) -> bass.Bass:
    """out[C, D] = xT.T @ wT (+ bB)   (xT: [D, C], wT: [D, D] both [contract, free])."""
    key = (C, with_bias)
    if key in _program_cache:
        return _program_cache[key]
    nc = bacc.Bacc("TRN2", target_bir_lowering=False, debug=False, num_devices=N_CORES)
    # Inputs declared fp32r so matmul operands come straight from DMA with no
    # on-device conversion pass (fp32r rounding happens in the PE datapath).
    xT = nc.dram_tensor("xT", [D, C], F32R, kind="ExternalInput")
    wT = nc.dram_tensor("wT", [D, D], F32R, kind="ExternalInput")
    bB = (
        nc.dram_tensor("bB", [P, D], F32, kind="ExternalInput")
        if with_bias
        else None
    )
    out = nc.dram_tensor("out", [C, D], F32, kind="ExternalOutput")

    n_ic = D // P  # contraction chunks
    n_ob = D // TT  # output blocks

    # Uniform 512-token tiles (last = remainder).
    t_tiles = []
    t0 = 0
    while t0 < C:
        tt = min(TT, C - t0)
        t_tiles.append((t0, tt))
        t0 += tt

    with tile.TileContext(nc) as tc:
        with (
            tc.tile_pool(name="wpool", bufs=1) as wpool,
            tc.tile_pool(name="bpool", bufs=1) as bpool,
            tc.tile_pool(name="xpool", bufs=4) as xpool,
            tc.tile_pool(name="opool", bufs=8) as opool,
            tc.tile_pool(name="psum", bufs=1, space="PSUM") as psum_pool,
        ):
            # Interleave x(t0)[ic] with w[ic] in ic order: the ic-outer matmul
            # schedule consumes exactly (x[ic], w[ic]) per stage, so the PE
            # starts after ~1MB of DMA and self-paces against the stream.
            first_tt = t_tiles[0][1]
            x_first = []
            w_tiles = []
            b_tile = None
            for ic in range(n_ic):
                xt = xpool.tile([P, first_tt], F32R, name=f"x_first{ic}", tag=f"x{ic}")
                nc.sync.dma_start(xt[:], xT[ic * P : (ic + 1) * P, :first_tt])
                x_first.append(xt)
                wt = wpool.tile([P, D], F32R, name=f"w{ic}", tag=f"w{ic}")
                nc.sync.dma_start(wt[:], wT[ic * P : (ic + 1) * P, :])
                w_tiles.append(wt)
                if with_bias and ic == 6:
                    b_tile = bpool.tile([P, D], F32, name="b_tile")
                    nc.sync.dma_start(b_tile[:], bB[:])

            for ti, (t0, tt) in enumerate(t_tiles):
                if ti == 0:
                    x_tiles = x_first
                else:
                    x_tiles = []
                    for ic in range(n_ic):
                        xt = xpool.tile([P, tt], F32R, tag=f"x{ic}")
                        nc.sync.dma_start(
                            xt[:], xT[ic * P : (ic + 1) * P, t0 : t0 + tt]
                        )
                        x_tiles.append(xt)
                # One psum bank per (token-chunk, output-block) group; run the
                # contraction ic-outer across all banks so each weight chunk
                # is needed only once per ~8 matmuls (JIT weight streaming).
                groups = [
                    (tc_, ob) for tc_ in range(tt // P) for ob in range(n_ob)
                ]
                ps = {
                    g: psum_pool.tile(
                        [P, TT], F32, name=f"ps_{ti}_{gi}", tag=f"ps{gi % 8}"
                    )
                    for gi, g in enumerate(groups)
                }
                for ic in range(n_ic):
                    for tc_, ob in groups:
                        nc.tensor.matmul(
                            ps[(tc_, ob)][:],
                            x_tiles[ic][:, tc_ * P : (tc_ + 1) * P],
                            w_tiles[ic][:, ob * TT : (ob + 1) * TT],
                            start=(ic == 0),
                            stop=(ic == n_ic - 1),
                        )
                for tc_, ob in groups:
                    ot = opool.tile([P, TT], F32)
                    if with_bias:
                        nc.vector.tensor_add(
                            ot[:], ps[(tc_, ob)][:], b_tile[:, ob * TT : (ob + 1) * TT]
                        )
                    else:
                        nc.vector.tensor_copy(ot[:], ps[(tc_, ob)][:])
                    nc.scalar.dma_start(
                        out[
                            t0 + tc_ * P : t0 + (tc_ + 1) * P,
                            ob * TT : (ob + 1) * TT,
                        ],
                        ot[:],
                    )
    nc.compile()
    _program_cache[key] = nc
    return nc


def kernel(actions, action_type, W, b, _trace=False):
    actions = np.ascontiguousarray(actions, dtype=np.float32)
    B, L, _ = actions.shape
    flat = actions.reshape(B * L, D)
    types = np.asarray(action_type).reshape(B * L).astype(np.int64)

    idx = [np.flatnonzero(types == t) for t in range(N_CORES)]
    counts = [len(i) for i in idx]
    # Cap device capacity at 2048 (4 perfectly uniform 512-token tiles);
    # rare overflow tokens are computed on the host instead.
    C = max(P, min(2048, -(-max(counts[1:]) // P) * P))

    W = np.asarray(W, dtype=np.float32)
    b_np = np.asarray(b, dtype=np.float32)
    eye = np.eye(D, dtype=np.float32)

    with_bias = bool(np.any(b_np))
    in_maps = []
    for t in range(N_CORES):
        n_dev = 0 if t == 0 else min(counts[t], C)
        xT = np.zeros((D, C), dtype=np.float32)
        if n_dev:
            xT[:, :n_dev] = flat[idx[t][:n_dev]].T
        wT = eye if t == 0 else np.ascontiguousarray(W[t - 1].T)
        m = {"xT": xT, "wT": wT}
        if with_bias:
            bvec = np.zeros(D, dtype=np.float32) if t == 0 else b_np[t - 1]
            m["bB"] = np.ascontiguousarray(np.broadcast_to(bvec, (P, D)))
        in_maps.append(m)

    nc = build_program(C, with_bias)
    r = run_bass_kernel_spmd(nc, in_maps, list(range(N_CORES)), trace=_trace)

    out_flat = np.empty_like(flat)
    out_flat[idx[0]] = flat[idx[0]]  # identity tokens: exact copy
    for t in range(1, N_CORES):
        n_dev = min(counts[t], C)
        if n_dev:
            out_flat[idx[t][:n_dev]] = r.results[t]["out"][:n_dev]
        if counts[t] > n_dev:  # overflow beyond device capacity: host BLAS
            ov = idx[t][n_dev:]
            out_flat[ov] = flat[ov] @ W[t - 1].T + b_np[t - 1]
    out = out_flat.reshape(B, L, D)
    if _trace:
        return out, r
    return out



# revision 3
# speedup vs baseline: 1.1423x; 1.1423x over previous
"""MoE routing kernel for Trainium2 (8 NeuronCores).

Problem: out[b,l,:] = actions[b,l,:]                      if action_type[b,l] == 0
         out[b,l,:] = W[t-1] @ actions[b,l,:] + b[t-1]    if action_type == t >= 1

Strategy (type-parallel, bf16): route each token to the single expert it
needs. The host groups the B*L tokens by action_type; core t (t=1..7)
processes the tokens of expert t-1 as one dense [C, D] x [D, D] matmul in
bf16 (rel RMSE ~3e-3, well under the 2e-2 gate). Identity-type tokens are
copied on the host (exact); core 0 runs the same SPMD program on zero
inputs and its output is discarded.

Token data is host-packed into SBUF-ready [128, free] bf16 layouts so the
device does a handful of large contiguous DMAs (vs dozens of small ones):
per 512-token tile, two 2MB-halved x loads; weights in two 1MB halves.
The PE runs 512-row bf16 matmuls back-to-back (~213ns each at full clock)
with all DMA hidden behind compute.
"""

import sys

for _p in ("/root/.axon_site/_ro/trn_rl_repo", "/opt/trn_rl_repo"):
    if _p not in sys.path:
        sys.path.append(_p)

import numpy as np
import ml_dtypes
import concourse.bass as bass
import concourse.tile as tile
from concourse import bacc, mybir
from concourse.bass_utils import run_bass_kernel_spmd

D = 1024
P = 128
N_CORES = 8
TT = 512  # token tile (psum-group block)
NIC = D // P  # 8 contraction chunks
NOB = D // TT  # 2 output feature blocks
F32 = mybir.dt.float32
BF16 = mybir.dt.bfloat16
BF16NP = ml_dtypes.bfloat16

_program_cache: dict[tuple, bass.Bass] = {}


def _t_tiles(C):
    tiles = []
    t0 = 0
    while t0 < C:
        tt = min(TT, C - t0)
        tiles.append((t0, tt))
        t0 += tt
    return tiles


def build_program(C: int, with_bias: bool) -> bass.Bass:
    """out[C, D] (bf16) = x @ w.T per-core, x/w host-packed bf16.

    DRAM inputs per core:
      xP [P, 8*C]  : block (ti, ic) at cols [(ti*8+ic)*tt ...] is
                     x.T[ic*128:(ic+1)*128, t0:t0+tt]   (contract chunk ic,
                     token tile ti) -- i.e. SBUF-ready, ic-major per tile.
      wP [P, 8*D]  : cols [ic*D ...] = w.T[ic*128:(ic+1)*128, :]
      bB [P, D]    : broadcast bias rows (only if with_bias)
    DRAM output: out [C, D] bf16, token-major.
    """
    key = (C, with_bias)
    if key in _program_cache:
        return _program_cache[key]
    nc = bacc.Bacc("TRN2", target_bir_lowering=False, debug=False, num_devices=N_CORES)
    xP = nc.dram_tensor("xP", [P, NIC * C], BF16, kind="ExternalInput")
    wP = nc.dram_tensor("wP", [P, NIC * D], BF16, kind="ExternalInput")
    bB = nc.dram_tensor("bB", [P, D], F32, kind="ExternalInput") if with_bias else None
    out = nc.dram_tensor("out", [C, D], BF16, kind="ExternalOutput")

    tiles = _t_tiles(C)

    with tile.TileContext(nc) as tc:
        with (
            tc.tile_pool(name="wpool", bufs=1) as wpool,
            tc.tile_pool(name="bpool", bufs=1) as bpool,
            tc.tile_pool(name="xpool", bufs=3) as xpool,
            tc.tile_pool(name="opool", bufs=2) as opool,
            tc.tile_pool(name="psum", bufs=1, space="PSUM") as psum_pool,
        ):
            HALF = NIC // 2  # ic chunks per x/w half-load
            # Preamble: wa + x0a first so the first matmul starts after
            # ~1.5MB of DMA, then the rest of the weights.
            w_half = []
            x0 = []
            tt0 = tiles[0][1]
            for h in range(2):
                wt = wpool.tile([P, HALF * D], BF16, name=f"w{h}", tag=f"w{h}")
                xt = xpool.tile([P, HALF * tt0], BF16, name=f"x0{h}", tag=f"x{h}")
                nc.sync.dma_start(wt[:], wP[:, h * HALF * D : (h + 1) * HALF * D])
                nc.sync.dma_start(
                    xt[:], xP[:, h * HALF * tt0 : (h + 1) * HALF * tt0]
                )
                w_half.append(wt)
                x0.append(xt)
            b_tile = None
            if with_bias:
                b_tile = bpool.tile([P, D], F32, name="b_tile")
                nc.sync.dma_start(b_tile[:], bB[:])

            for ti, (t0, tt) in enumerate(tiles):
                ntc = tt // P  # token chunks in this tile
                if ti == 0:
                    xh = x0
                else:
                    xh = []
                    base = NIC * t0
                    for h in range(2):
                        xt = xpool.tile([P, HALF * tt], BF16, tag=f"x{h}")
                        nc.sync.dma_start(
                            xt[:],
                            xP[:, base + h * HALF * tt : base + (h + 1) * HALF * tt],
                        )
                        xh.append(xt)

                ps = {
                    (c, ob): psum_pool.tile(
                        [P, TT], F32, name=f"ps_{ti}_{c}_{ob}", tag=f"ps{c}_{ob}"
                    )
                    for c in range(ntc)
                    for ob in range(NOB)
                }
                for ic in range(NIC):
                    h, ici = divmod(ic, HALF)
                    for c in range(ntc):
                        lhsT = xh[h][:, ici * tt + c * P : ici * tt + (c + 1) * P]
                        for ob in range(NOB):
                            nc.tensor.matmul(
                                ps[(c, ob)][:],
                                lhsT,
                                w_half[h][:, ici * D + ob * TT : ici * D + (ob + 1) * TT],
                                start=(ic == 0),
                                stop=(ic == NIC - 1),
                            )
                for c in range(ntc):
                    ot = opool.tile([P, D], BF16, tag=f"o{c}")
                    for ob in range(NOB):
                        if with_bias:
                            nc.vector.tensor_add(
                                ot[:, ob * TT : (ob + 1) * TT],
                                ps[(c, ob)][:],
                                b_tile[:, ob * TT : (ob + 1) * TT],
                            )
                        else:
                            nc.vector.tensor_copy(
                                ot[:, ob * TT : (ob + 1) * TT], ps[(c, ob)][:]
                            )
                    nc.scalar.dma_start(
                        out[t0 + c * P : t0 + (c + 1) * P, :], ot[:]
                    )
    nc.compile()
    _program_cache[key] = nc
    return nc


def _pack_x(flat_rows: np.ndarray, C: int) -> np.ndarray:
    """[n, D] fp32 tokens -> [P, NIC*C] bf16 in (ti, ic)-block layout."""
    n = flat_rows.shape[0]
    xT = np.zeros((D, C), dtype=np.float32)
    if n:
        xT[:, :n] = flat_rows.T
    xP = np.empty((P, NIC * C), dtype=BF16NP)
    for ti, (t0, tt) in enumerate(_t_tiles(C)):
        base = NIC * t0
        for ic in range(NIC):
            xP[:, base + ic * tt : base + (ic + 1) * tt] = xT[
                ic * P : (ic + 1) * P, t0 : t0 + tt
            ].astype(BF16NP)
    return xP


def kernel(actions, action_type, W, b, _trace=False):
    actions = np.ascontiguousarray(actions, dtype=np.float32)
    B, L, _ = actions.shape
    flat = actions.reshape(B * L, D)
    types = np.asarray(action_type).reshape(B * L).astype(np.int64)

    idx = [np.flatnonzero(types == t) for t in range(N_CORES)]
    counts = [len(i) for i in idx]
    # Cap device capacity at 2048 (4 uniform 512-token tiles); rare
    # overflow tokens beyond that are computed on the host instead.
    C = max(P, min(2048, -(-max(counts[1:]) // P) * P))

    W = np.asarray(W, dtype=np.float32)
    b_np = np.asarray(b, dtype=np.float32)

    with_bias = bool(np.any(b_np))
    # wP: [P, NIC*D] bf16, cols [ic*D:(ic+1)*D] = w.T[ic*P:(ic+1)*P, :]
    in_maps = []
    for t in range(N_CORES):
        n_dev = 0 if t == 0 else min(counts[t], C)
        rows = flat[idx[t][:n_dev]] if n_dev else np.zeros((0, D), np.float32)
        wT = np.eye(D, dtype=np.float32) if t == 0 else W[t - 1].T
        wP = np.empty((P, NIC * D), dtype=BF16NP)
        for ic in range(NIC):
            wP[:, ic * D : (ic + 1) * D] = wT[ic * P : (ic + 1) * P, :].astype(BF16NP)
        m = {"xP": _pack_x(rows, C), "wP": wP}
        if with_bias:
            bvec = np.zeros(D, dtype=np.float32) if t == 0 else b_np[t - 1]
            m["bB"] = np.ascontiguousarray(
                np.broadcast_to(bvec, (P, D)), dtype=np.float32
            )
        in_maps.append(m)

    nc = build_program(C, with_bias)
    r = run_bass_kernel_spmd(nc, in_maps, list(range(N_CORES)), trace=_trace)

    out_flat = np.empty_like(flat)
    out_flat[idx[0]] = flat[idx[0]]  # identity tokens: exact copy
    for t in range(1, N_CORES):
        n_dev = min(counts[t], C)
        if n_dev:
            out_flat[idx[t][:n_dev]] = (
                r.results[t]["out"][:n_dev].astype(np.float32)
            )
        if counts[t] > n_dev:  # overflow beyond device capacity: host BLAS
            ov = idx[t][n_dev:]
            out_flat[ov] = flat[ov] @ W[t - 1].T + b_np[t - 1]
    out = out_flat.reshape(B, L, D)
    if _trace:
        return out, r
    return out


# revision 6
# speedup vs baseline: 1.1480x; 1.0050x over previous
"""MoE routing kernel for Trainium2 (8 NeuronCores).

Problem: out[b,l,:] = actions[b,l,:]                      if action_type[b,l] == 0
         out[b,l,:] = W[t-1] @ actions[b,l,:] + b[t-1]    if action_type == t >= 1

Strategy (type-parallel, bf16): route each token to the single expert it
needs. The host groups the B*L tokens by action_type; core t (t=1..7)
processes the tokens of expert t-1 as one dense [C, D] x [D, D] matmul in
bf16 (rel RMSE ~3e-3, well under the 2e-2 gate). Identity-type tokens are
copied on the host (exact); core 0 runs the same SPMD program on zero
inputs and its output is discarded.

Device schedule: 256-token tiles, each = 4 PSUM groups [128 tok, 512 feat]
accumulated over 8 contraction chunks; two alternating 4-bank PSUM sets so
tile i+1's matmuls never wait on tile i's PSUM->SBUF casts. Host packs x/w
into SBUF-ready [128, free] bf16 blocks so all DMAs are large and
contiguous; the first weight/x chunks are split fine (ic0 alone) so the
first matmul starts ~1.3us after DMA begins instead of ~7us.
"""

import sys

for _p in ("/root/.axon_site/_ro/trn_rl_repo", "/opt/trn_rl_repo"):
    if _p not in sys.path:
        sys.path.append(_p)

import numpy as np
import ml_dtypes
import concourse.bass as bass
import concourse.tile as tile
from concourse import bacc, mybir
from concourse.bass_utils import run_bass_kernel_spmd

D = 1024
P = 128
N_CORES = 8
TT = 256  # token tile
FB = 512  # psum feature block
NIC = D // P  # 8 contraction chunks
NOB = D // FB  # 2 output feature blocks
F32 = mybir.dt.float32
BF16 = mybir.dt.bfloat16
BF16NP = ml_dtypes.bfloat16

_program_cache: dict[tuple, bass.Bass] = {}


def _t_tiles(C):
    tiles = []
    t0 = 0
    while t0 < C:
        tt = min(TT, C - t0)
        tiles.append((t0, tt))
        t0 += tt
    return tiles


def build_program(C: int, with_bias: bool) -> bass.Bass:
    """out[C, D] (bf16) = x @ w.T per-core, x/w host-packed bf16.

    DRAM inputs per core:
      xP [P, 8*C]  : cols [(NIC*t0 + ic*tt) ...] hold
                     x.T[ic*128:(ic+1)*128, t0:t0+tt]  (contract chunk ic,
                     token tile [t0, t0+tt)) -- SBUF-ready, ic-major per tile.
      wP [P, 8*D]  : cols [ic*D ...] = w.T[ic*128:(ic+1)*128, :]
      bB [P, D]    : broadcast bias rows (only if with_bias)
    DRAM output: out [C, D] bf16, token-major.
    """
    key = (C, with_bias)
    if key in _program_cache:
        return _program_cache[key]
    nc = bacc.Bacc("TRN2", target_bir_lowering=False, debug=False, num_devices=N_CORES)
    xP = nc.dram_tensor("xP", [P, NIC * C], BF16, kind="ExternalInput")
    wP = nc.dram_tensor("wP", [P, NIC * D], BF16, kind="ExternalInput")
    bB = nc.dram_tensor("bB", [P, D], F32, kind="ExternalInput") if with_bias else None
    out = nc.dram_tensor("out", [C, D], BF16, kind="ExternalOutput")

    tiles = _t_tiles(C)
    # Weight chunk split (in ic units): ic0 alone so the first matmul's
    # operands are small, then growing chunks.
    W_CHUNKS = [(0, 1), (1, 1), (2, 2), (4, 4)]

    with tile.TileContext(nc) as tc:
        with (
            tc.tile_pool(name="wpool", bufs=1) as wpool,
            tc.tile_pool(name="bpool", bufs=1) as bpool,
            tc.tile_pool(name="xpool", bufs=2) as xpool,
            tc.tile_pool(name="opool", bufs=2) as opool,
            tc.tile_pool(name="psum", bufs=1, space="PSUM") as psum_pool,
        ):
            tt0 = tiles[0][1]
            # Preamble order: w[ic0], x0[ic0], then the rest interleaved.
            w_tiles = [None] * NIC  # per-ic view (tile, col offset)
            x0_tiles = [None] * NIC
            X0_CHUNKS = [(0, 1), (1, 3), (4, 4)]

            def _dma_w(ic0_, nic_):
                wt = wpool.tile([P, nic_ * D], BF16, name=f"w{ic0_}", tag=f"w{ic0_}")
                nc.sync.dma_start(wt[:], wP[:, ic0_ * D : (ic0_ + nic_) * D])
                for j in range(nic_):
                    w_tiles[ic0_ + j] = (wt, j * D)

            def _dma_x0(ic0_, nic_):
                xt = xpool.tile(
                    [P, nic_ * tt0], BF16, name=f"x0_{ic0_}", tag=f"x0_{ic0_}"
                )
                nc.sync.dma_start(
                    xt[:], xP[:, ic0_ * tt0 : (ic0_ + nic_) * tt0]
                )
                for j in range(nic_):
                    x0_tiles[ic0_ + j] = (xt, j * tt0)

            _dma_w(*W_CHUNKS[0])
            _dma_x0(*X0_CHUNKS[0])
            _dma_w(*W_CHUNKS[1])
            _dma_x0(*X0_CHUNKS[1])
            _dma_w(*W_CHUNKS[2])
            _dma_x0(*X0_CHUNKS[2])
            _dma_w(*W_CHUNKS[3])
            b_tile = None
            if with_bias:
                b_tile = bpool.tile([P, D], F32, name="b_tile")
                nc.sync.dma_start(b_tile[:], bB[:])

            for ti, (t0, tt) in enumerate(tiles):
                ntc = tt // P  # token chunks in this tile (2 for full tiles)
                if ti == 0:
                    xv = x0_tiles
                else:
                    xt = xpool.tile([P, NIC * tt], BF16, tag=f"x{ti % 3}")
                    nc.sync.dma_start(
                        xt[:], xP[:, NIC * t0 : NIC * (t0 + tt)]
                    )
                    xv = [(xt, ic * tt) for ic in range(NIC)]

                par = ti % 2  # alternate psum bank set
                ps = {
                    (c, ob): psum_pool.tile(
                        [P, FB], F32, name=f"ps_{ti}_{c}_{ob}", tag=f"ps{par}_{c}_{ob}"
                    )
                    for c in range(ntc)
                    for ob in range(NOB)
                }
                ots = {}
                for ic in range(NIC):
                    last = ic == NIC - 1
                    for c in range(ntc):
                        xt, xoff = xv[ic]
                        lhsT = xt[:, xoff + c * P : xoff + (c + 1) * P]
                        if last:
                            ots[c] = opool.tile(
                                [P, D], BF16, name=f"ot_{ti}_{c}", tag=f"o{c}"
                            )
                        for ob in range(NOB):
                            wt, woff = w_tiles[ic]
                            nc.tensor.matmul(
                                ps[(c, ob)][:],
                                lhsT,
                                wt[:, woff + ob * FB : woff + (ob + 1) * FB],
                                start=(ic == 0),
                                stop=last,
                            )
                            if last:
                                # evacuate psum as soon as its group closes
                                if with_bias:
                                    nc.vector.tensor_add(
                                        ots[c][:, ob * FB : (ob + 1) * FB],
                                        ps[(c, ob)][:],
                                        b_tile[:, ob * FB : (ob + 1) * FB],
                                    )
                                else:
                                    nc.vector.tensor_copy(
                                        ots[c][:, ob * FB : (ob + 1) * FB],
                                        ps[(c, ob)][:],
                                    )
                for c in range(ntc):
                    nc.scalar.dma_start(
                        out[t0 + c * P : t0 + (c + 1) * P, :], ots[c][:]
                    )
    nc.compile()
    _program_cache[key] = nc
    return nc


def _pack_x(flat_rows: np.ndarray, C: int) -> np.ndarray:
    """[n, D] fp32 tokens -> [P, NIC*C] bf16 in (tile, ic)-block layout."""
    n = flat_rows.shape[0]
    xT = np.zeros((D, C), dtype=np.float32)
    if n:
        xT[:, :n] = flat_rows.T
    xP = np.empty((P, NIC * C), dtype=BF16NP)
    for t0, tt in _t_tiles(C):
        base = NIC * t0
        for ic in range(NIC):
            xP[:, base + ic * tt : base + (ic + 1) * tt] = xT[
                ic * P : (ic + 1) * P, t0 : t0 + tt
            ].astype(BF16NP)
    return xP


def kernel(actions, action_type, W, b, _trace=False):
    actions = np.ascontiguousarray(actions, dtype=np.float32)
    B, L, _ = actions.shape
    flat = actions.reshape(B * L, D)
    types = np.asarray(action_type).reshape(B * L).astype(np.int64)

    idx = [np.flatnonzero(types == t) for t in range(N_CORES)]
    counts = [len(i) for i in idx]
    # Cap device capacity at 2048 (8 uniform 256-token tiles); rare
    # overflow tokens beyond that are computed on the host instead.
    C = max(P, min(2048, -(-max(counts[1:]) // P) * P))

    W = np.asarray(W, dtype=np.float32)
    b_np = np.asarray(b, dtype=np.float32)

    with_bias = bool(np.any(b_np))
    in_maps = []
    for t in range(N_CORES):
        n_dev = 0 if t == 0 else min(counts[t], C)
        rows = flat[idx[t][:n_dev]] if n_dev else np.zeros((0, D), np.float32)
        wT = np.eye(D, dtype=np.float32) if t == 0 else W[t - 1].T
        wP = np.empty((P, NIC * D), dtype=BF16NP)
        for ic in range(NIC):
            wP[:, ic * D : (ic + 1) * D] = wT[ic * P : (ic + 1) * P, :].astype(BF16NP)
        m = {"xP": _pack_x(rows, C), "wP": wP}
        if with_bias:
            bvec = np.zeros(D, dtype=np.float32) if t == 0 else b_np[t - 1]
            m["bB"] = np.ascontiguousarray(
                np.broadcast_to(bvec, (P, D)), dtype=np.float32
            )
        in_maps.append(m)

    nc = build_program(C, with_bias)
    r = run_bass_kernel_spmd(nc, in_maps, list(range(N_CORES)), trace=_trace)

    out_flat = np.empty_like(flat)
    out_flat[idx[0]] = flat[idx[0]]  # identity tokens: exact copy
    for t in range(1, N_CORES):
        n_dev = min(counts[t], C)
        if n_dev:
            out_flat[idx[t][:n_dev]] = (
                r.results[t]["out"][:n_dev].astype(np.float32)
            )
        if counts[t] > n_dev:  # overflow beyond device capacity: host BLAS
            ov = idx[t][n_dev:]
            out_flat[ov] = flat[ov] @ W[t - 1].T + b_np[t - 1]
    out = out_flat.reshape(B, L, D)
    if _trace:
        return out, r
    return out
